# revision 24
# baseline (speedup 1.0000x reference)
"""AEG-Conv2d Trainium2 kernel (8 NeuronCores, data-parallel over batch).

Math: the reference's 9-step scan  r <- (r+x)*y / (r+y)*x  (parity of i+j+k)
unrolls to  r = sum_k a_k * prod_{j>=k} m_j, which factors per output-pixel
parity into  out[n,oc,px] = sum_{t=0..8} sum_ic U_t[n,ic,px] * V_t[oc,ic]:
a 288-deep contraction where U_t are products of shifted input patches
(computed on-chip) and V_t are products of weight taps (computed on host).

Layout (v2): instead of shipping 18 pre-gathered 512-px strips (2.25 MB,
~9 us of input DMA inside the measured window), the host ships a compact
checkerboard-deinterleaved double-plane image D [128 part = rg*32+ic,
2*18*66] with the two color blocks of each row ordered by ROW parity:
D[p, v*1188 + r*66 + b*33 + c] = xpad[ic, 16rg+r, 2c+(b+r)%2+v] (xpad
zero-padded to (66,67)).  Every shifted tap strip in the (se,m) pixel
order is then a 2-free-dim affine view of D:
    addr(se, m) = base(par,dh,dw) + 66*se + m,   se = 2s+e in 0..15
with a CONTIGUOUS inner m-run of 32 (keeps the DVE 2x_1p fast path and
the walrus 3D-AP limit), and every chain-mate tap pair [A|B] is one
3-free-dim AP with a constant pair stride (into the v=1 plane for odd
column shifts).  Input DMA: 0.6 MB instead of 2.6 MB.

Feature chain (DVE, 9 tensor_muls: 6 paired + 3 single) and matmuls
(72, K=32 row-tiled at partition base 32*rg, M=64, tile_position
(32rg, 64par), accumulating 9 taps into PSUM bank rg) are interleaved
wave-by-wave so matmul waves fire as their features land.

Timing notes (profile-driven):
- The measured exec window covers the whole program from the first
  instruction to roughly the last output-DMA activity; the framework
  preamble (~6.5 us of firmware boot + barriers + register loads before
  any kernel instruction can run) is fixed, so the levers are the input
  DMA (volume + which HWDGE ring: SP moves ~250 GB/s, Act only ~82),
  the DVE chain, and the evac/output tail.
- Input DMAs are hoisted into the prologue; the framework's const-AP
  memsets are deleted (nothing references them).
- A dummy early ACTIVATE pins ACT_TABLE_LOAD (1.3 us) off the critical
  path; it is otherwise emitted lazily right before the evac ACTIVATEs.
- All feature muls stay on DVE: concurrent GPSIMD tensor ops slow DVE
  ~3x (measured 420ns -> 1300ns per op, SBUF port sharing).
- Output in two SP-ring DMAs (banks 0-1 as soon as their evacs land,
  banks 2-3 after): tile deps are range-precise, so half the output
  streams ~1us early at no trigger-latency cost.
- Measured: ~17.4 us (from 27.3 us baseline); rel err 3.2e-3.
"""

import numpy as np

IC, OC, H, W = 32, 64, 64, 64
N = 8
RG = 4           # row groups per core
PLANE = 18 * 66  # 1188: one deinterleaved padded plane
XFREE = 2 * PLANE
VFREE = 9 * 2 * OC           # 1152
OUTFREE = RG * 512           # 2048

MM_DTYPE = "bfloat16"        # matmul input dtype (1 PE cycle/row; fp32 would be 4)


def _base_of(par, t):
    """Base offset of tap strip (par,dh,dw) in D; the strip is then the
    2-free-dim view  addr(se, m) = base + 66*se + m,  se=2s+e in 0..15."""
    dh, dw = divmod(t, 3)
    v = dw & 1                       # odd col shifts read the v=1 plane
    b0 = (par + dh + dw - v) % 2
    c0 = (dw - v) // 2
    return v * PLANE + dh * 66 + b0 * 33 + c0


# chain ops in DVE emission order:
#   ("pair", par, (ta, tb), src)  -> tile [U_ta | U_tb]
#   ("one",  par, t, src)         -> tile [U_t]
# op k multiplies tap-strip views (from D) by src: ("root", par, t) = raw
# strip view of D, or ("feat", par, t) = previously computed feature.
_CHAIN = [
    ("pair", 0, (5, 6), ("root", 0, 7)),
    ("pair", 1, (6, 7), ("root", 1, 8)),
    ("pair", 0, (3, 4), ("feat", 0, 5)),
    ("pair", 1, (4, 5), ("feat", 1, 6)),
    ("pair", 0, (1, 2), ("feat", 0, 3)),
    ("pair", 1, (2, 3), ("feat", 1, 4)),
    ("one", 0, 0, ("feat", 0, 1)),
    ("one", 1, 1, ("feat", 1, 2)),
    ("one", 1, 0, ("feat", 1, 2)),
]
_ROOTS = {(0, 7), (0, 8), (1, 8)}       # raw strip views (no feature tile)

# matmul waves: emitted in feature-availability order.  Each entry is a
# list of (par, tap); every wave spans all 4 row groups.
_WAVES = [
    [(0, 7), (0, 8), (1, 8)],           # roots: ready at DMA-land
    [(0, 5), (0, 6)],
    [(1, 6), (1, 7)],
    [(0, 3), (0, 4)],
    [(1, 4), (1, 5)],
    [(0, 1), (0, 2)],
    [(1, 2), (1, 3)],
    [(0, 0)],
    [(1, 1)],
    [(1, 0)],
]
_FIRST = {(0, 7), (1, 8)}               # psum accumulation start per parity
_LAST = {(0, 0), (1, 0)}                # psum accumulation stop per parity


def _bass_modules():
    import sys
    try:
        import concourse.bass as bass
    except ImportError:
        sys.path.insert(0, "/opt/trn_rl_repo")
        import concourse.bass as bass
    import concourse.mybir as mybir
    import concourse.tile as tile
    from concourse import bass_utils
    return bass, mybir, tile, bass_utils


def _delete_const_memsets(nc, mybir):
    """Drop the framework's const-AP registration memsets from the prologue.
    Safe only while no instruction references the const tiles -- asserted."""
    f = nc.m.functions[0]
    main = f.blocks[0]
    memsets = [i for i in main.instructions
               if type(i).__name__ == "InstMemset"]
    victims = set()
    for i in memsets:
        assert not (i.sync_info and list(i.sync_info.on_wait)), i
        assert not (i.sync_info and list(i.sync_info.on_update)), i
        for o in i.outs:
            victims.add(str(o.memref))
    assert all(v.startswith("const-") for v in victims), victims
    for b in f.blocks:
        for i in b.instructions:
            if type(i).__name__ == "InstMemset":
                continue
            for a in list(i.ins) + list(i.outs):
                mr = getattr(a, "memref", None)
                assert mr is None or str(mr) not in victims, (i, mr)
    main.instructions[:] = [i for i in main.instructions
                            if type(i).__name__ != "InstMemset"]


def _hoist_input_dmas(nc, mybir):
    """Move the input-load DMACopy triggers (no waits, SP engine) from the
    body block into the prologue block, before SP enters the all-engine
    barrier -- the loads then overlap the other engines' preamble."""
    f = nc.m.functions[0]
    blocks = list(f.blocks)
    pro, body = blocks[0], blocks[1]
    moved = []
    bil = body.instructions
    i = 0
    while i < len(bil):
        inst = bil[i]
        si = inst.sync_info
        if (type(inst).__name__ == "InstDMACopy"
                and (si is None or not list(si.on_wait))):
            moved.append(inst)    # input loads are the only wait-free DMAs
            del bil[i]
            continue
        i += 1
    pil = pro.instructions
    for inst in moved:
        idx = None
        for j, p in enumerate(pil):
            if type(p).__name__ == "InstDrain" and p.engine == inst.engine:
                idx = j
                break
        if idx is None:
            idx = len(pil) - 5
        pil.insert(idx, inst)


def _split_multi_waits(nc, mybir, limit=1):
    """walrus codegen in this toolchain allows only one sync-wait command per
    engine instruction; hoist surplus waits into standalone InstEventSemaphore
    instructions inserted just before, on the same engine queue."""
    eng_of_sem = {
        "PE_": mybir.EngineType.PE, "DVE_": mybir.EngineType.DVE,
        "Activation_": mybir.EngineType.Activation,
        "Pool_": mybir.EngineType.Pool,
    }
    n = 0
    for f in nc.m.functions:
        for b in f.blocks:
            il = b.instructions
            i = 0
            while i < len(il):
                inst = il[i]
                si = inst.sync_info
                if si is not None and len(si.on_wait) > limit:
                    waits = list(si.on_wait)
                    for w in waits[:-limit]:
                        ev = mybir.InstEventSemaphore(
                            name=f"wsplit_{n}", ins=[], outs=[])
                        n += 1
                        ev.engine = inst.engine
                        if type(inst).__name__ == "InstDrain":
                            for pfx, eng in eng_of_sem.items():
                                if w.ant_name.startswith(pfx) and eng != inst.engine:
                                    ev.engine = eng
                                    break
                        ev.sync_info = mybir.SyncInfo(on_wait=[w], on_update=[])
                        il.insert(i, ev)
                        i += 1
                    inst.sync_info = mybir.SyncInfo(
                        on_wait=waits[-limit:], on_update=list(si.on_update))
                i += 1


def build_nc(hw_opt=True):
    bass, mybir, tile, _ = _bass_modules()
    F32 = mybir.dt.float32
    mmdt = getattr(mybir.dt, MM_DTYPE)
    BF16 = mybir.dt.bfloat16
    nc = bass.Bass()
    x_in = nc.declare_dram_parameter("xs", [128, XFREE], BF16, isOutput=False)
    v_in = nc.declare_dram_parameter("vtab", [128, VFREE], mmdt, isOutput=False)
    out_ext = nc.declare_dram_parameter("out", [128, OUTFREE], BF16, isOutput=True)

    with tile.TileContext(nc) as tc:
        with tc.tile_pool(name="sb", bufs=1) as pool, \
             tc.tile_pool(name="ps", bufs=1, space="PSUM") as pp:
            # xd first on the SP ring: it gates the feature chain; vtab
            # (only gates the slack-rich matmul waves) follows.  (Routing
            # any input over the Act ring measured consistently ~1-2us
            # WORSE end-to-end: that ring moves ~82 GB/s vs SP's ~250.)
            xd = pool.tile([128, XFREE], BF16, tag="xd")
            nc.sync.dma_start(xd[:], x_in[:])
            vt = pool.tile([128, VFREE], mmdt, tag="vt")
            nc.sync.dma_start(vt[:], v_in[:])
            # dummy Act op: forces walrus to emit ACT_TABLE_LOAD at the top
            # of the Act queue (it is emitted lazily before the first
            # ACTIVATE, and would otherwise land on the critical path
            # behind the evac waits, costing ~1.3us).  Reads vt (already
            # DMA-initialized) so CoreSim sees no uninitialized access.
            scratch = pool.tile([128, 1], F32, tag="scratch")
            nc.scalar.copy(scratch[:, 0:1], vt[0:128, 0:1])

            def _view(p0, pcnt, offset, dims):
                """AP over xd: partitions [p0,p0+pcnt), given free dims."""
                v = xd[p0:p0 + pcnt, offset:offset + 1].copy()
                pstep = tuple(list(v.ap)[0])[0]
                v.ap = mybir.VecI64Pair([(pstep, pcnt)] + list(dims))
                return v

            def strip(par, t, rg=None, paired_with=None, bcast=False):
                base = _base_of(par, t)
                dims = [(66, 16), (1, 32)]
                if paired_with is not None:
                    dims = [(_base_of(par, paired_with) - base, 2)] + dims
                if bcast:
                    dims = [(0, 2)] + dims
                p0, cnt = (0, 128) if rg is None else (32 * rg, 32)
                return _view(p0, cnt, base, dims)

            def bcast_feat(ap_owner, width=512):
                """read the same 512-px block twice: [(0,2),(1,512)]"""
                v = ap_owner.copy()
                pstep = tuple(list(v.ap)[0])[0]
                cnt = tuple(list(v.ap)[0])[1]
                v.ap = mybir.VecI64Pair([(pstep, cnt), (0, 2), (1, width)])
                return v

            feats = {}   # (par, t) -> AP of its 512 block

            def srcap(src, paired):
                if src[0] == "root":
                    return strip(src[1], src[2], bcast=paired)
                base = feats[(src[1], src[2])]
                return bcast_feat(base) if paired else base

            for op in _CHAIN:
                if op[0] == "pair":
                    _, par, (ta, tb), src = op
                    ft = pool.tile([128, 1024], mmdt, tag=f"f{par}_{ta}{tb}")
                    nc.vector.tensor_mul(
                        ft[:], strip(par, ta, paired_with=tb),
                        srcap(src, True))
                    feats[(par, ta)] = ft[:, 0:512]
                    feats[(par, tb)] = ft[:, 512:1024]
                else:
                    _, par, t, src = op
                    ft = pool.tile([128, 512], mmdt, tag=f"f{par}_{t}")
                    nc.vector.tensor_mul(ft[:], strip(par, t),
                                         srcap(src, False))
                    feats[(par, t)] = ft[:]

            # 4 psum banks (separate tiles: per-bank dep tracking lets each
            # bank's evac start as soon as ITS matmuls finish)
            psums = [pp.tile([128, 512], F32, tag=f"acc{rg}", name=f"acc{rg}")
                     for rg in range(RG)]
            outb = pool.tile([128, OUTFREE], BF16, tag="outb")
            for wave in _WAVES:
                for par, t in wave:
                    for rg in range(RG):
                        if (par, t) in _ROOTS:
                            rhs = strip(par, t, rg=rg)
                        else:
                            f = feats[(par, t)]
                            rhs = f[32 * rg:32 * (rg + 1), :]
                        lhsT = vt[32 * rg:32 * (rg + 1),
                                  t * 128 + 64 * par: t * 128 + 64 * par + 64]
                        nc.tensor.matmul(
                            psums[rg][64 * par:64 * par + 64, :],
                            lhsT=lhsT, rhs=rhs,
                            start=((par, t) in _FIRST),
                            stop=((par, t) in _LAST),
                            skip_group_check=True,
                            tile_position=(32 * rg, 64 * par))
            # per-bank evac on ScalarE + VectorE in parallel (banks finish
            # in rg order), then ONE output trigger on the idle SP ring
            evac_eng = [nc.scalar.copy, nc.vector.tensor_copy,
                        nc.scalar.copy, nc.vector.tensor_copy]
            for rg in range(RG):
                evac_eng[rg](outb[:, 512 * rg:512 * (rg + 1)], psums[rg][:])
                if rg == 1:
                    # range-precise deps: this waits only banks 0-1, so
                    # half the output data streams ~1us earlier
                    nc.sync.dma_start(out_ext[:, 0:1024], outb[:, 0:1024])
            nc.sync.dma_start(out_ext[:, 1024:2048], outb[:, 1024:2048])
    if hw_opt:
        _delete_const_memsets(nc, mybir)
        _hoist_input_dmas(nc, mybir)
        _split_multi_waits(nc, mybir)
    return nc


def host_inputs(x, weight):
    y = weight.reshape(OC, IC, 9).transpose(2, 0, 1).astype(np.float64)
    V = np.empty_like(y)
    V[8] = y[8]; V[6] = y[6] * V[8]; V[4] = y[4] * V[6]; V[2] = y[2] * V[4]
    V[0] = y[0] * V[2]; V[1] = y[1] * V[2]; V[3] = y[3] * V[4]; V[5] = y[5] * V[6]
    V[7] = y[7] * V[8]
    Vo = np.empty_like(y)
    Vo[7] = y[7]; Vo[5] = y[5] * Vo[7]; Vo[3] = y[3] * Vo[5]; Vo[1] = y[1] * Vo[3]
    Vo[0] = y[0] * Vo[1]; Vo[2] = y[2] * Vo[3]; Vo[4] = y[4] * Vo[5]; Vo[6] = y[6] * Vo[7]
    Vo[8] = y[8]
    import ml_dtypes
    vt = np.stack([V, Vo], 1)                                   # (9, 2, OC, IC)
    vflat = vt.transpose(3, 0, 1, 2).reshape(IC, VFREE)
    vtab = np.ascontiguousarray(
        np.tile(vflat, (RG, 1)).astype(ml_dtypes.bfloat16))     # (128, 1152)

    # checkerboard-deinterleaved double-plane image, blocks ordered by row
    # parity: D[rg*32+ic, v*1188 + r*66 + b*33 + c] = xpad[ic, 16rg+r,
    # 2c+(b+r)%2+v] -- every shifted tap strip is then a 2-free-dim view.
    rg_i = np.arange(RG)[:, None, None, None, None]
    v_i = np.arange(2)[None, :, None, None, None]
    r_i = np.arange(18)[None, None, :, None, None]
    b_i = np.arange(2)[None, None, None, :, None]
    c_i = np.arange(33)[None, None, None, None, :]
    rows = (16 * rg_i + r_i) + 0 * (v_i + b_i + c_i)
    cols = (2 * c_i + (b_i + r_i) % 2 + v_i) + 0 * rg_i
    xss = []
    for i in range(x.shape[0]):
        xp = np.pad(x[i], ((0, 0), (1, 1), (1, 2)))     # (32, 66, 67)
        D = xp[:, rows, cols].transpose(1, 0, 2, 3, 4, 5).reshape(128, XFREE)
        xss.append(np.ascontiguousarray(D.astype(ml_dtypes.bfloat16)))
    return xss, vtab


_RGI = np.arange(RG)[:, None, None, None]
_SI = np.arange(8)[None, :, None, None]
_EI = np.arange(2)[None, None, :, None]
_MI = np.arange(32)[None, None, None, :]
_ROWS = 16 * _RGI + 2 * _SI + _EI
_COLE = 2 * _MI + _EI
_COLO = 2 * _MI + 1 - _EI


def decode_out(out):
    o = np.asarray(out, dtype=np.float32).reshape(2, OC, RG, 8, 2, 32)
    full = np.empty((OC, H, W), np.float32)
    full[:, _ROWS, _COLE] = o[0]
    full[:, _ROWS, _COLO] = o[1]
    return full


def kernel(x, weight):
    _, _, _, bass_utils = _bass_modules()
    x = np.ascontiguousarray(np.asarray(x), dtype=np.float32)
    weight = np.ascontiguousarray(np.asarray(weight), dtype=np.float32)
    xss, vtab = host_inputs(x, weight)
    nc = build_nc()
    in_maps = [{"xs": xss[i], "vtab": vtab} for i in range(N)]
    res = bass_utils.run_bass_kernel_spmd(nc, in_maps, core_ids=list(range(N)))
    return np.stack([decode_out(res.results[i]["out"]) for i in range(N)], 0)
